# revision 34
# baseline (speedup 1.0000x reference)
"""Multi-head attention (B=4, S=2048, E=1024, H=16, D=64) on 8 TRN2 NeuronCores.

Sharding: data-parallel over batch (4) x sequence-parallel over queries (2).
Core c handles batch c//2 and query half c%2 (1024 queries), ALL 16 heads.
K/V are computed redundantly per core from the full (replicated) x[b], so no
cross-core reduction or collective is needed anywhere.  The host rotates the
key sequence for odd cores (swap halves) so each core's queries are always
columns 0..1023 of its xt — the program stays SPMD-uniform, and attention is
permutation-invariant over keys.

Weights/biases are identical on every core, so they are baked into the NEFF
as Const tensors (DMA'd to HBM once at model load).  The only per-execute
I/O is xt [1024, 2048] bf16 in and y^T [1024, 1024] f32 out — per-execute
dispatch cost through the PJRT path scales with I/O bytes/buffers.

Per-core dataflow (matmuls bf16 inputs, fp32 PSUM accumulation):
  phase 1: QKV projection.
    Q^T [1024, 1024] / K^T [1024, 2048] column-major via lhsT=W, rhs=x^T.
    V [2048, 16, 64] row-major via lhsT=x^T, rhs=Wv; a bias row appended to
    Wv plus an on-chip ones row implements +bias; an on-chip ones *column*
    appended to V makes the PV matmul also produce softmax row-sums.
  phase 2: per head: S^T = K^T-tiles @ Q^T (keys on partitions), exp on
    ScalarE straight from PSUM (fused 1/8 scale, bf16 out), O^T[65,1024]
    accumulated over 16 key-blocks where row 64 = softmax denominator l.
    Normalize: partition-broadcast DMA of 1/l, DVE multiply (fused bf16).
  phase 3: out-proj y^T[1024,1024] = Wo-tiles^T @ O^T (+bias), DMA to HBM.
"""

import numpy as np
import ml_dtypes

B, S, E, H, D = 4, 2048, 1024, 16, 64
NCORES = 8
SQ = S // 2            # queries per core = 1024
P = 128
KT = E // P            # 8 contraction tiles over E
SB = S // P            # 16 key blocks of 128
PAIRS = H // 2         # 8 head pairs (2 heads share a 128-row block)
QC = SQ // 512         # 2 query chunks of 512
KC = S // 512          # 4 key-dim chunks of 512 (for K^T projection)

_BF16 = ml_dtypes.bfloat16

_cached = {}


_MULTIWAIT_OK = ("InstMatmult", "InstActivation", "InstTensor", "InstCopy",
                 "InstMemSet", "InstReciprocal", "InstLdweights")


def _split_drain_waits(nc, mybir, max_waits=1, compute_max_waits=1):
    """This walrus build rejects >1 sem wait per instruction (setupSyncWait
    S3D3_TS fires even for DVE/ACT ops); hoist extras onto preceding
    same-engine nops."""
    for f in nc.m.functions:
        for bb in f.blocks:
            insts = bb.instructions
            i = 0
            while i < len(insts):
                inst = insts[i]
                si = inst.sync_info
                lim = (
                    compute_max_waits
                    if type(inst).__name__.startswith(_MULTIWAIT_OK)
                    else max_waits
                )
                if si is not None and len(si.on_wait) > lim:
                    extra = list(si.on_wait[lim:])
                    keep = list(si.on_wait[:lim])
                    for j, w in enumerate(extra):
                        nop = mybir.InstNoOp(
                            name=f"{inst.name}-waitsplit{j}", ins=[], outs=[]
                        )
                        nop.engine = inst.engine
                        nop.sync_info = mybir.SyncInfo(on_wait=[w], on_update=[])
                        nc.register_instruction(nop)
                        insts.insert(i, nop)
                        i += 1
                    inst.sync_info = mybir.SyncInfo(
                        on_wait=keep, on_update=list(si.on_update)
                    )
                i += 1


def _weight_consts(Wqkv, bqkv, Wo, bo):
    """Host-side packing of the (core-invariant) weight constants."""
    Wqkv = np.asarray(Wqkv, np.float32)
    bqkv = np.asarray(bqkv, np.float32)
    Wo = np.asarray(Wo, np.float32)
    bo = np.asarray(bo, np.float32)

    wqk = np.ascontiguousarray(Wqkv[:, 0:2 * E]).astype(_BF16)   # [E, 2E] Wq|Wk
    wv = np.empty((E + 1, E), _BF16)                             # Wv + bias row
    wv[0:E] = Wqkv[:, 2 * E:3 * E].astype(_BF16)
    wv[E] = bqkv[2 * E:3 * E].astype(_BF16)
    wo = Wo.astype(_BF16)                                        # [CS, E]
    bias = np.empty((P, 24), np.float32)
    bias[:, 0:8] = bqkv[0:E].reshape(8, P).T                     # Q bias blocks
    bias[:, 8:16] = bqkv[E:2 * E].reshape(8, P).T                # K bias blocks
    bias[:, 16:24] = bo.reshape(8, P).T                          # out bias blocks
    return wqk, wv, wo, bias


def _build_program(consts):
    import os as _os
    import concourse.bass as bass
    import concourse.tile as tile
    from concourse import mybir

    f32 = mybir.dt.float32
    bf16 = mybir.dt.bfloat16

    wqk_np, wv_np, wo_np, bias_np = consts

    nc = bass.Bass("TRN2", num_devices=NCORES, debug=False)

    xt_d = nc.dram_tensor("xt", [E, S], bf16, kind="ExternalInput")
    out_d = nc.dram_tensor("out", [E, SQ], bf16, kind="ExternalOutput")
    wqk_d = nc.inline_tensor(wqk_np, name="wqk_c")
    wv_d = nc.inline_tensor(wv_np, name="wv_c")
    wo_d = nc.inline_tensor(wo_np, name="wo_c")
    bias_d = nc.inline_tensor(bias_np, name="bias_c")

    _taps = _os.environ.get("K_TAPS") == "1"
    if _taps:
        tap_q = nc.dram_tensor("tap_q", [P, SQ], bf16, kind="ExternalOutput")
        tap_k = nc.dram_tensor("tap_k", [P, S], bf16, kind="ExternalOutput")
        tap_v = nc.dram_tensor("tap_v", [P, H * (D + 1)], bf16, kind="ExternalOutput")
        tap_on = nc.dram_tensor("tap_on", [P, SQ], bf16, kind="ExternalOutput")

    with tile.TileContext(nc) as tc:
        with (
            tc.tile_pool(name="const", bufs=1) as const,
            tc.tile_pool(name="xtw", bufs=1) as xtw,
            tc.tile_pool(name="wvon", bufs=1) as wvon,
            tc.tile_pool(name="qk", bufs=1) as qkp,
            tc.tile_pool(name="vsb", bufs=1) as vp,
            tc.tile_pool(name="pt", bufs=int(__import__("os").environ.get("K_PT", "4"))) as ptp,
            tc.tile_pool(name="rec", bufs=int(__import__("os").environ.get("K_REC", "2"))) as recp,
            tc.tile_pool(name="ysb", bufs=2) as yp,
            tc.tile_pool(name="dram", bufs=1, space="DRAM") as dram,
        ):
          _repeat = int(_os.environ.get("K_REPEAT", "1"))
          _order = _os.environ.get("K_ORDER", "fillv")
          for _rep in range(_repeat):
            # ---- constants / weights into SBUF ----
            bias_sb = const.tile([P, 24], f32, tag="bias")
            nc.sync.dma_start(out=bias_sb[:], in_=bias_d.ap())
            ones_sb = const.tile([1, S], bf16, tag="ones")
            nc.vector.memset(ones_sb[:], 1.0)

            # x^T [128, 2048] and [Wq|Wk] [128, 2048] tiles, interleaved so
            # the first QK chains can start as soon as their tiles land
            xt_sb = [
                xtw.tile([P, S], bf16, tag=f"xt{k}", name=f"xt{_rep}_{k}")
                for k in range(KT)
            ]
            wqk_sb = [
                xtw.tile([P, S], bf16, tag=f"wqk{k}", name=f"wqk{_rep}_{k}")
                for k in range(KT)
            ]
            for k in range(KT):
                nc.sync.dma_start(out=xt_sb[k][:], in_=xt_d[k * P:(k + 1) * P, :])
                nc.sync.dma_start(out=wqk_sb[k][:], in_=wqk_d[k * P:(k + 1) * P, :])

            wv_sb = [
                wvon.tile([P, E], bf16, tag=f"wv{k}", name=f"wv{_rep}_{k}")
                for k in range(KT)
            ]
            for k in range(KT):
                nc.sync.dma_start(out=wv_sb[k][:], in_=wv_d[k * P:(k + 1) * P, :])
            wvb_sb = const.tile([1, E], bf16, tag="wvb")
            nc.sync.dma_start(out=wvb_sb[:], in_=wv_d[E:E + 1, :])

            # Wo tiles reuse the xt slots (xt dead once the last QK chain ran)
            wo_sb = [
                xtw.tile([P, E], bf16, tag=f"xt{p}", name=f"wo{_rep}_{p}")
                for p in range(PAIRS)
            ]
            for p in range(PAIRS):
                nc.sync.dma_start(out=wo_sb[p][:], in_=wo_d[p * P:(p + 1) * P, :])

            # persistent activations
            qkq_sb = [
                qkp.tile([P, SQ], bf16, tag=f"qq{c}", name=f"qq{_rep}_{c}")
                for c in range(PAIRS)
            ]
            qkk_sb = [
                qkp.tile([P, S], bf16, tag=f"qk{c}", name=f"qk{_rep}_{c}")
                for c in range(PAIRS)
            ]
            v_sb = [
                vp.tile([P, H, D + 1], bf16, tag=f"v{s}", name=f"v{_rep}_{s}")
                for s in range(SB)
            ]
            # O^T (normalized) tiles reuse the wv pool slots (wv is dead once
            # the V phase finishes; same [128, 1024] bf16 footprint)
            on_sb = [
                wvon.tile([P, SQ], bf16, tag=f"wv{p}", name=f"on{_rep}_{p}")
                for p in range(PAIRS)
            ]

            # ---- shared PSUM pools (8 banks static across all phases):
            # pair mode: work 2x1-bank + acc 3x2-bank;
            # head mode: work 4x1-bank + acc 2x2-bank ----
            _pair = _os.environ.get("K_PAIR", "0") == "1"
            work_cm = tc.tile_pool(name="work", bufs=2 if _pair else 4, space="PSUM")
            work = work_cm.__enter__()
            acc_cm = tc.tile_pool(name="acc", bufs=3 if _pair else 2, space="PSUM")
            acc = acc_cm.__enter__()

            def v_chain(s):
                # V: out[key-block s, 1024 head-dims]; lhsT = x^T tile, rhs = Wv
                def chain():
                    for half in range(2):
                        o = half * 512
                        pv = work.tile(
                            [P, 512], f32, tag="w", name=f"pv{_rep}_{s}_{half}"
                        )
                        for k in range(KT):
                            nc.tensor.matmul(
                                pv[:],
                                xt_sb[k][:, s * P:(s + 1) * P],
                                wv_sb[k][:, o:o + 512],
                                start=(k == 0),
                                stop=False,
                            )
                        nc.tensor.matmul(
                            pv[:],
                            ones_sb[:, s * P:(s + 1) * P],
                            wvb_sb[:, o:o + 512],
                            start=False,
                            stop=True,
                        )
                        if half == 0:
                            nc.vector.memset(v_sb[s][:, :, D:D + 1], 1.0)
                        nc.vector.tensor_copy(
                            v_sb[s][:, half * 8:(half + 1) * 8, 0:D], pv[:]
                        )
                return chain

            def q_chain(c, q):
                def chain():
                    pq = work.tile([P, 512], f32, tag="w", name=f"pq{_rep}_{c}_{q}")
                    for k in range(KT):
                        nc.tensor.matmul(
                            pq[:],
                            wqk_sb[k][:, c * P:(c + 1) * P],
                            xt_sb[k][:, q * 512:(q + 1) * 512],
                            start=(k == 0),
                            stop=(k == KT - 1),
                        )
                    nc.vector.tensor_scalar_add(
                        qkq_sb[c][:, q * 512:(q + 1) * 512],
                        pq[:],
                        bias_sb[:, c:c + 1],
                    )
                return chain

            def k_chain(c, q):
                def chain():
                    pq = work.tile([P, 512], f32, tag="w", name=f"pk{_rep}_{c}_{q}")
                    for k in range(KT):
                        nc.tensor.matmul(
                            pq[:],
                            wqk_sb[k][:, E + c * P:E + (c + 1) * P],
                            xt_sb[k][:, q * 512:(q + 1) * 512],
                            start=(k == 0),
                            stop=(k == KT - 1),
                        )
                    nc.vector.tensor_scalar_add(
                        qkk_sb[c][:, q * 512:(q + 1) * 512],
                        pq[:],
                        bias_sb[:, 8 + c:9 + c],
                    )
                return chain

            def qk_pair_chains(p):
                return [q_chain(p, q) for q in range(QC)] + [
                    k_chain(p, q) for q in range(KC)
                ]

            def emit_qk_pair(p):
                for ch in qk_pair_chains(p):
                    ch()

            def emit_head(h, fillers=(), stride=2):
                # Software-pipelined by one key block: kb's scores+exp are
                # emitted before kb-1's PV matmuls, so the PE never stalls
                # waiting for ScalarE (the PV of kb-1 reads an exp result
                # that finished while kb's scores streamed).
                fillers = list(fillers)
                p, half = h // 2, h % 2
                r0 = half * D
                qT = qkq_sb[p]
                kT = qkk_sb[p]
                po = acc.tile([D + 1, SQ], f32, tag="a", name=f"po{_rep}_{h}")

                def emit_pv(pt_prev, kb_prev):
                    for q2 in range(QC):
                        nc.tensor.matmul(
                            po[:, q2 * 512:(q2 + 1) * 512],
                            v_sb[kb_prev][:, h, :],
                            pt_prev[:, q2 * 512:(q2 + 1) * 512],
                            start=(kb_prev == 0),
                            stop=(kb_prev == SB - 1),
                        )

                prev = None
                for kb in range(SB):
                    if fillers and (stride == 1 or kb % stride == 1):
                        fillers.pop(0)()
                    pt = ptp.tile([P, SQ], bf16, tag="pt", name=f"pt{_rep}_{h}_{kb}")
                    pss = []
                    for q2 in range(QC):
                        ps = work.tile(
                            [P, 512], f32, tag="w", name=f"ps{_rep}_{h}_{kb}_{q2}"
                        )
                        nc.tensor.matmul(
                            ps[:],
                            kT[r0:r0 + D, kb * P:(kb + 1) * P],
                            qT[r0:r0 + D, q2 * 512:(q2 + 1) * 512],
                            start=True,
                            stop=True,
                        )
                        pss.append(ps)
                    for q2 in range(QC):
                        nc.scalar.activation(
                            pt[:, q2 * 512:(q2 + 1) * 512],
                            pss[q2][:],
                            mybir.ActivationFunctionType.Exp,
                            scale=0.125,
                        )
                    if prev is not None:
                        emit_pv(*prev)
                    prev = (pt, kb)
                emit_pv(*prev)
                for ch in fillers:
                    ch()
                # normalize: O^T[0:D] / l (l = row D), write bf16
                lsb = recp.tile([1, SQ], f32, tag="lsb", name=f"lsb{_rep}_{h}")
                nc.vector.reciprocal(lsb[:], po[D:D + 1, :])
                lscr = dram.tile([1, SQ], f32, tag="lscr", name=f"lscr{_rep}_{h}", bufs=2)
                nc.sync.dma_start(out=lscr[:], in_=lsb[:])
                ldr = lscr[:]
                lbc = bass.AP(
                    tensor=ldr.tensor,
                    offset=ldr.offset,
                    ap=[[0, D]] + [list(x) for x in ldr.ap[1:]],
                )
                rin = recp.tile([D, SQ], f32, tag="rin", name=f"rin{_rep}_{h}")
                nc.sync.dma_start(out=rin[:], in_=lbc)
                nc.vector.tensor_mul(
                    on_sb[p][r0:r0 + D, :],
                    po[0:D, :],
                    rin[:],
                )

            def emit_pair(p, fillers=(), stride=2):
                # Heads (2p, 2p+1) interleaved per key-block: their K=64 score
                # matmuls sit at base partitions 0 and 64, so the PE runs them
                # concurrently in disjoint row-groups (tile_position
                # auto-derives from the lhsT base partition).
                fillers = list(fillers)
                qT = qkq_sb[p]
                kT = qkk_sb[p]
                po = [
                    acc.tile([D + 1, SQ], f32, tag="a", name=f"po{_rep}_{p}_{i}")
                    for i in range(2)
                ]
                for kb in range(SB):
                    if fillers and (stride == 1 or kb % stride == 1):
                        fillers.pop(0)()
                    pt = [
                        ptp.tile([P, SQ], bf16, tag="pt", name=f"pt{_rep}_{p}_{kb}_{i}")
                        for i in range(2)
                    ]
                    for q2 in range(QC):
                        pss = []
                        for i in range(2):
                            r0 = i * D
                            ps = work.tile(
                                [P, 512], f32, tag="w",
                                name=f"ps{_rep}_{p}_{kb}_{q2}_{i}",
                            )
                            nc.tensor.matmul(
                                ps[:],
                                kT[r0:r0 + D, kb * P:(kb + 1) * P],
                                qT[r0:r0 + D, q2 * 512:(q2 + 1) * 512],
                                start=True,
                                stop=True,
                            )
                            pss.append(ps)
                        for i in range(2):
                            nc.scalar.activation(
                                pt[i][:, q2 * 512:(q2 + 1) * 512],
                                pss[i][:],
                                mybir.ActivationFunctionType.Exp,
                                scale=0.125,
                            )
                        for i in range(2):
                            nc.tensor.matmul(
                                po[i][:, q2 * 512:(q2 + 1) * 512],
                                v_sb[kb][:, 2 * p + i, :],
                                pt[i][:, q2 * 512:(q2 + 1) * 512],
                                start=(kb == 0),
                                stop=(kb == SB - 1),
                            )
                for ch in fillers:
                    ch()
                # normalize: O^T[0:D] / l (l = row D), write bf16
                for i in range(2):
                    r0 = i * D
                    lsb = recp.tile([1, SQ], f32, tag="lsb", name=f"lsb{_rep}_{p}_{i}")
                    nc.vector.reciprocal(lsb[:], po[i][D:D + 1, :])
                    lscr = dram.tile(
                        [1, SQ], f32, tag="lscr", name=f"lscr{_rep}_{p}_{i}", bufs=2
                    )
                    nc.sync.dma_start(out=lscr[:], in_=lsb[:])
                    ldr = lscr[:]
                    lbc = bass.AP(
                        tensor=ldr.tensor,
                        offset=ldr.offset,
                        ap=[[0, D]] + [list(x) for x in ldr.ap[1:]],
                    )
                    rin = recp.tile([D, SQ], f32, tag="rin", name=f"rin{_rep}_{p}_{i}")
                    nc.sync.dma_start(out=rin[:], in_=lbc)
                    nc.vector.tensor_mul(
                        on_sb[p][r0:r0 + D, :],
                        po[i][0:D, :],
                        rin[:],
                    )

            if _pair:
                # QK pair 0 first so ScalarE starts ASAP; the 16 V chains ride
                # as per-kb fillers inside pair 0 (v_sb[kb] lands just in time
                # for pair 0's PV at kb); pair p also hosts pair p+1's QK
                # chains as fillers.
                emit_qk_pair(0)
                emit_pair(
                    0,
                    fillers=[v_chain(s) for s in range(SB)] + qk_pair_chains(1),
                    stride=1,
                )
                for p in range(1, PAIRS):
                    emit_pair(p, fillers=qk_pair_chains(p + 1) if p + 1 < PAIRS else ())
            elif _order == "fillv":
                emit_qk_pair(0)
                emit_head(
                    0,
                    fillers=[v_chain(s) for s in range(SB)],
                    stride=1,
                )
                emit_head(1, fillers=qk_pair_chains(1))
                for p in range(1, PAIRS - 1):
                    emit_head(2 * p)
                    emit_head(2 * p + 1, fillers=qk_pair_chains(p + 1))
                emit_head(2 * PAIRS - 2)
                emit_head(2 * PAIRS - 1)
            else:  # "safe"
                for ch in [v_chain(s) for s in range(SB)]:
                    ch()
                emit_qk_pair(0)
                emit_head(0)
                emit_head(1, fillers=qk_pair_chains(1))
                for p in range(1, PAIRS - 1):
                    emit_head(2 * p)
                    emit_head(2 * p + 1, fillers=qk_pair_chains(p + 1))
                emit_head(2 * PAIRS - 2)
                emit_head(2 * PAIRS - 1)

            # ---- phase 3: output projection (one DMA per e-block) ----
            for e in range(E // P):
                ye = yp.tile([P, SQ], bf16, tag="ysb", name=f"ye{_rep}_{e}")
                for q in range(QC):
                    py = work.tile([P, 512], f32, tag="w", name=f"py{_rep}_{e}_{q}")
                    for p in range(PAIRS):
                        nc.tensor.matmul(
                            py[:],
                            wo_sb[p][:, e * P:(e + 1) * P],
                            on_sb[p][:, q * 512:(q + 1) * 512],
                            start=(p == 0),
                            stop=(p == PAIRS - 1),
                        )
                    nc.vector.tensor_scalar_add(
                        ye[:, q * 512:(q + 1) * 512], py[:], bias_sb[:, 16 + e:17 + e]
                    )
                nc.sync.dma_start(out=out_d[e * P:(e + 1) * P, :], in_=ye[:])

            if _taps:
                nc.sync.dma_start(out=tap_q.ap(), in_=qkq_sb[0][:])
                nc.sync.dma_start(out=tap_k.ap(), in_=qkk_sb[0][:])
                nc.sync.dma_start(
                    out=tap_v.ap(),
                    in_=v_sb[0].rearrange("p h d -> p (h d)"),
                )
                nc.sync.dma_start(out=tap_on.ap(), in_=on_sb[0][:])

            acc_cm.__exit__(None, None, None)
            work_cm.__exit__(None, None, None)

    _split_drain_waits(nc, mybir)
    return nc


def _host_shards(x, Wqkv=None, bqkv=None, Wo=None, bo=None):
    """Per-core inputs: only xt (x[b]^T, key-halves swapped on odd cores)."""
    x = np.asarray(x, np.float32)
    in_maps = []
    for c in range(NCORES):
        b, h = c // 2, c % 2
        xt = np.ascontiguousarray(x[b].T).astype(_BF16)
        if h == 1:
            xt = np.concatenate([xt[:, SQ:], xt[:, :SQ]], axis=1)
        in_maps.append({"xt": np.ascontiguousarray(xt)})
    return in_maps


def _get_runner(consts):
    """Build the Bass program once (weights baked in as NEFF constants) and
    wrap it in a cached 8-core jitted callable.  Rebuilds if the weights
    change (the constants are baked into the NEFF)."""
    import zlib
    key = tuple(
        (a.shape, a.dtype.str, zlib.crc32(np.ascontiguousarray(a).view(np.uint8)))
        for a in consts
    )
    if "runner" in _cached and _cached.get("consts_key") == key:
        return _cached["runner"]
    _cached.pop("runner", None)
    _cached["consts_key"] = key

    import jax
    from jax.sharding import Mesh, PartitionSpec, NamedSharding
    from jax.experimental.shard_map import shard_map
    from concourse import bass2jax, mybir

    nc = _build_program(consts)
    _cached["nc"] = nc
    bass2jax.install_neuronx_cc_hook()

    partition_name = nc.partition_id_tensor.name if nc.partition_id_tensor else None
    in_names, out_names, out_avals = [], [], []
    for alloc in nc.m.functions[0].allocations:
        if not isinstance(alloc, mybir.MemoryLocationSet):
            continue
        if alloc.kind not in ("ExternalInput", "ExternalOutput"):
            continue
        name = alloc.memorylocations[0].name
        if alloc.kind == "ExternalInput":
            if name != partition_name:
                in_names.append(name)
        elif alloc.kind == "ExternalOutput":
            out_names.append(name)
            out_avals.append(
                jax.core.ShapedArray(tuple(alloc.tensor_shape), mybir.dt.np(alloc.dtype))
            )
    n_params = len(in_names)
    all_in_names = list(in_names) + list(out_names)
    if partition_name is not None:
        all_in_names.append(partition_name)

    def _body(*args):
        operands = list(args)
        if partition_name is not None:
            operands.append(bass2jax.partition_id_tensor())
        outs = bass2jax._bass_exec_p.bind(
            *operands,
            out_avals=tuple(out_avals),
            in_names=tuple(all_in_names),
            out_names=tuple(out_names),
            lowering_input_output_aliases=(),
            sim_require_finite=True,
            sim_require_nnan=True,
            nc=nc,
        )
        return tuple(outs)

    import os as _os
    devices = jax.devices()[:NCORES]
    mesh = Mesh(np.asarray(devices), ("core",))
    in_specs = (PartitionSpec("core"),) * (n_params + len(out_names))
    out_specs = (PartitionSpec("core"),) * len(out_names)
    sharding = NamedSharding(mesh, PartitionSpec("core"))
    zero_shapes = [
        ((NCORES * a.shape[0],) + tuple(a.shape[1:]), a.dtype) for a in out_avals
    ]
    donate = _os.environ.get("K_DONATE", "1") == "1"
    fastd = _os.environ.get("K_FASTD", "1") == "1"
    sharded = shard_map(
        _body, mesh=mesh, in_specs=in_specs, out_specs=out_specs, check_rep=False
    )
    donate_argnums = (
        tuple(range(n_params, n_params + len(out_names))) if donate else ()
    )
    if fastd:
        # lower with abstract avals so the compile happens inside the
        # fast-dispatch context (C++ no-effect dispatch path)
        in_sds = []
        for alloc_name in in_names:
            for alloc in nc.m.functions[0].allocations:
                if not isinstance(alloc, mybir.MemoryLocationSet):
                    continue
                if alloc.memorylocations[0].name == alloc_name:
                    shp = tuple(alloc.tensor_shape)
                    in_sds.append(
                        jax.ShapeDtypeStruct(
                            (NCORES * shp[0],) + shp[1:],
                            mybir.dt.np(alloc.dtype),
                            sharding=sharding,
                        )
                    )
                    break
        for shp, dt in zero_shapes:
            in_sds.append(jax.ShapeDtypeStruct(shp, dt, sharding=sharding))
        jitted = bass2jax.fast_dispatch_compile(
            lambda: jax.jit(
                sharded, donate_argnums=donate_argnums, keep_unused=True
            ).lower(*in_sds).compile()
        )
    else:
        jitted = jax.jit(
            sharded, donate_argnums=donate_argnums, keep_unused=True
        )

    def run(in_maps):
        concat_in = [
            np.concatenate([np.asarray(in_maps[c][nm]) for c in range(NCORES)], axis=0)
            for nm in in_names
        ]
        args = [jax.device_put(a, sharding) for a in concat_in] + [
            jax.device_put(np.zeros(shp, dt), sharding) for shp, dt in zero_shapes
        ]
        outs = jitted(*args)
        outs = [np.asarray(o) for o in outs]
        per_core = [
            {
                nm: outs[i].reshape(NCORES, *out_avals[i].shape)[c]
                for i, nm in enumerate(out_names)
            }
            for c in range(NCORES)
        ]
        return per_core

    _cached["runner"] = run
    _cached["jitted"] = jitted
    _cached["meta"] = (in_names, out_names, out_avals, sharding)
    _cached["donate"] = donate
    return run


def kernel(x, Wqkv, bqkv, Wo, bo):
    run = _get_runner(_weight_consts(Wqkv, bqkv, Wo, bo))
    in_maps = _host_shards(x)
    results = run(in_maps)

    out = np.empty((B, S, E), np.float32)
    for c in range(NCORES):
        b, h = c // 2, c % 2
        out[b, h * SQ:(h + 1) * SQ, :] = results[c]["out"].T
    return out


# revision 35
# speedup vs baseline: 2.2874x; 2.2874x over previous
"""Multi-head attention (B=4, S=2048, E=1024, H=16, D=64) on 8 TRN2 NeuronCores.

Sharding: data-parallel over batch (4) x sequence-parallel over queries (2).
Core c handles batch c//2 and query half c%2 (1024 queries), ALL 16 heads.
K/V are computed redundantly per core from the full (replicated) x[b], so no
cross-core reduction or collective is needed anywhere.  The host rotates the
key sequence for odd cores (swap halves) so each core's queries are always
columns 0..1023 of its xt — the program stays SPMD-uniform, and attention is
permutation-invariant over keys.

Weights/biases are identical on every core, so they are baked into the NEFF
as Const tensors (DMA'd to HBM once at model load).  The only per-execute
I/O is xt [1024, 2048] bf16 in and y^T [1024, 1024] f32 out — per-execute
dispatch cost through the PJRT path scales with I/O bytes/buffers.

Per-core dataflow (matmuls bf16 inputs, fp32 PSUM accumulation):
  phase 1: QKV projection.
    Q^T [1024, 1024] / K^T [1024, 2048] column-major via lhsT=W, rhs=x^T.
    V [2048, 16, 64] row-major via lhsT=x^T, rhs=Wv; a bias row appended to
    Wv plus an on-chip ones row implements +bias; an on-chip ones *column*
    appended to V makes the PV matmul also produce softmax row-sums.
  phase 2: per head: S^T = K^T-tiles @ Q^T (keys on partitions), exp on
    ScalarE straight from PSUM (fused 1/8 scale, bf16 out), O^T[65,1024]
    accumulated over 16 key-blocks where row 64 = softmax denominator l.
    Normalize: partition-broadcast DMA of 1/l, DVE multiply (fused bf16).
  phase 3: out-proj y^T[1024,1024] = Wo-tiles^T @ O^T (+bias), DMA to HBM.
"""

import numpy as np
import ml_dtypes

B, S, E, H, D = 4, 2048, 1024, 16, 64
NCORES = 8
SQ = S // 2            # queries per core = 1024
P = 128
KT = E // P            # 8 contraction tiles over E
SB = S // P            # 16 key blocks of 128
PAIRS = H // 2         # 8 head pairs (2 heads share a 128-row block)
QC = SQ // 512         # 2 query chunks of 512
KC = S // 512          # 4 key-dim chunks of 512 (for K^T projection)

_BF16 = ml_dtypes.bfloat16

_cached = {}


_MULTIWAIT_OK = ("InstMatmult", "InstActivation", "InstTensor", "InstCopy",
                 "InstMemSet", "InstReciprocal", "InstLdweights")


def _split_drain_waits(nc, mybir, max_waits=1, compute_max_waits=1):
    """This walrus build rejects >1 sem wait per instruction (setupSyncWait
    S3D3_TS fires even for DVE/ACT ops); hoist extras onto preceding
    same-engine nops."""
    for f in nc.m.functions:
        for bb in f.blocks:
            insts = bb.instructions
            i = 0
            while i < len(insts):
                inst = insts[i]
                si = inst.sync_info
                lim = (
                    compute_max_waits
                    if type(inst).__name__.startswith(_MULTIWAIT_OK)
                    else max_waits
                )
                if si is not None and len(si.on_wait) > lim:
                    extra = list(si.on_wait[lim:])
                    keep = list(si.on_wait[:lim])
                    for j, w in enumerate(extra):
                        nop = mybir.InstNoOp(
                            name=f"{inst.name}-waitsplit{j}", ins=[], outs=[]
                        )
                        nop.engine = inst.engine
                        nop.sync_info = mybir.SyncInfo(on_wait=[w], on_update=[])
                        nc.register_instruction(nop)
                        insts.insert(i, nop)
                        i += 1
                    inst.sync_info = mybir.SyncInfo(
                        on_wait=keep, on_update=list(si.on_update)
                    )
                i += 1


def _weight_consts(Wqkv, bqkv, Wo, bo):
    """Host-side packing of the (core-invariant) weight constants."""
    Wqkv = np.asarray(Wqkv, np.float32)
    bqkv = np.asarray(bqkv, np.float32)
    Wo = np.asarray(Wo, np.float32)
    bo = np.asarray(bo, np.float32)

    wqk = np.ascontiguousarray(Wqkv[:, 0:2 * E]).astype(_BF16)   # [E, 2E] Wq|Wk
    wv = np.empty((E + 1, E), _BF16)                             # Wv + bias row
    wv[0:E] = Wqkv[:, 2 * E:3 * E].astype(_BF16)
    wv[E] = bqkv[2 * E:3 * E].astype(_BF16)
    wo = Wo.astype(_BF16)                                        # [CS, E]
    bias = np.empty((P, 24), np.float32)
    bias[:, 0:8] = bqkv[0:E].reshape(8, P).T                     # Q bias blocks
    bias[:, 8:16] = bqkv[E:2 * E].reshape(8, P).T                # K bias blocks
    bias[:, 16:24] = bo.reshape(8, P).T                          # out bias blocks
    return wqk, wv, wo, bias


def _build_program(consts):
    import os as _os
    import concourse.bass as bass
    import concourse.tile as tile
    from concourse import mybir

    f32 = mybir.dt.float32
    bf16 = mybir.dt.bfloat16

    wqk_np, wv_np, wo_np, bias_np = consts

    nc = bass.Bass("TRN2", num_devices=NCORES, debug=False)

    xt_d = nc.dram_tensor("xt", [E, S], bf16, kind="ExternalInput")
    out_d = nc.dram_tensor("out", [E, SQ], bf16, kind="ExternalOutput")
    wqk_d = nc.inline_tensor(wqk_np, name="wqk_c")
    wv_d = nc.inline_tensor(wv_np, name="wv_c")
    wo_d = nc.inline_tensor(wo_np, name="wo_c")
    bias_d = nc.inline_tensor(bias_np, name="bias_c")

    _taps = _os.environ.get("K_TAPS") == "1"
    if _taps:
        tap_q = nc.dram_tensor("tap_q", [P, SQ], bf16, kind="ExternalOutput")
        tap_k = nc.dram_tensor("tap_k", [P, S], bf16, kind="ExternalOutput")
        tap_v = nc.dram_tensor("tap_v", [P, H * (D + 1)], bf16, kind="ExternalOutput")
        tap_on = nc.dram_tensor("tap_on", [P, SQ], bf16, kind="ExternalOutput")

    with tile.TileContext(nc) as tc:
        with (
            tc.tile_pool(name="const", bufs=1) as const,
            tc.tile_pool(name="xtw", bufs=1) as xtw,
            tc.tile_pool(name="wvon", bufs=1) as wvon,
            tc.tile_pool(name="qk", bufs=1) as qkp,
            tc.tile_pool(name="vsb", bufs=1) as vp,
            tc.tile_pool(name="pt", bufs=int(__import__("os").environ.get("K_PT", "4"))) as ptp,
            tc.tile_pool(name="rec", bufs=int(__import__("os").environ.get("K_REC", "2"))) as recp,
            tc.tile_pool(name="ysb", bufs=2) as yp,
            tc.tile_pool(name="dram", bufs=1, space="DRAM") as dram,
        ):
          _repeat = int(_os.environ.get("K_REPEAT", "1"))
          _order = _os.environ.get("K_ORDER", "fillv")
          for _rep in range(_repeat):
            # ---- constants / weights into SBUF ----
            bias_sb = const.tile([P, 24], f32, tag="bias")
            nc.sync.dma_start(out=bias_sb[:], in_=bias_d.ap())
            ones_sb = const.tile([1, S], bf16, tag="ones")
            nc.vector.memset(ones_sb[:], 1.0)

            # x^T [128, 2048] and [Wq|Wk] [128, 2048] tiles, interleaved so
            # the first QK chains can start as soon as their tiles land
            xt_sb = [
                xtw.tile([P, S], bf16, tag=f"xt{k}", name=f"xt{_rep}_{k}")
                for k in range(KT)
            ]
            wqk_sb = [
                xtw.tile([P, S], bf16, tag=f"wqk{k}", name=f"wqk{_rep}_{k}")
                for k in range(KT)
            ]
            for k in range(KT):
                nc.sync.dma_start(out=xt_sb[k][:], in_=xt_d[k * P:(k + 1) * P, :])
                nc.sync.dma_start(out=wqk_sb[k][:], in_=wqk_d[k * P:(k + 1) * P, :])

            wv_sb = [
                wvon.tile([P, E], bf16, tag=f"wv{k}", name=f"wv{_rep}_{k}")
                for k in range(KT)
            ]
            for k in range(KT):
                nc.sync.dma_start(out=wv_sb[k][:], in_=wv_d[k * P:(k + 1) * P, :])
            wvb_sb = const.tile([1, E], bf16, tag="wvb")
            nc.sync.dma_start(out=wvb_sb[:], in_=wv_d[E:E + 1, :])

            # Wo tiles reuse the xt slots (xt dead once the last QK chain ran)
            wo_sb = [
                xtw.tile([P, E], bf16, tag=f"xt{p}", name=f"wo{_rep}_{p}")
                for p in range(PAIRS)
            ]
            for p in range(PAIRS):
                nc.sync.dma_start(out=wo_sb[p][:], in_=wo_d[p * P:(p + 1) * P, :])

            # persistent activations
            qkq_sb = [
                qkp.tile([P, SQ], bf16, tag=f"qq{c}", name=f"qq{_rep}_{c}")
                for c in range(PAIRS)
            ]
            qkk_sb = [
                qkp.tile([P, S], bf16, tag=f"qk{c}", name=f"qk{_rep}_{c}")
                for c in range(PAIRS)
            ]
            v_sb = [
                vp.tile([P, H, D + 1], bf16, tag=f"v{s}", name=f"v{_rep}_{s}")
                for s in range(SB)
            ]
            # O^T (normalized) tiles reuse the wv pool slots (wv is dead once
            # the V phase finishes; same [128, 1024] bf16 footprint)
            on_sb = [
                wvon.tile([P, SQ], bf16, tag=f"wv{p}", name=f"on{_rep}_{p}")
                for p in range(PAIRS)
            ]

            # ---- shared PSUM pools (8 banks static across all phases):
            # pair mode: work 2x1-bank + acc 3x2-bank;
            # head mode: work 4x1-bank + acc 2x2-bank ----
            _pair = _os.environ.get("K_PAIR", "0") == "1"
            work_cm = tc.tile_pool(name="work", bufs=2 if _pair else 4, space="PSUM")
            work = work_cm.__enter__()
            acc_cm = tc.tile_pool(name="acc", bufs=3 if _pair else 2, space="PSUM")
            acc = acc_cm.__enter__()

            def v_chain(s):
                # V: out[key-block s, 1024 head-dims]; lhsT = x^T tile, rhs = Wv
                def chain():
                    for half in range(2):
                        o = half * 512
                        pv = work.tile(
                            [P, 512], f32, tag="w", name=f"pv{_rep}_{s}_{half}"
                        )
                        for k in range(KT):
                            nc.tensor.matmul(
                                pv[:],
                                xt_sb[k][:, s * P:(s + 1) * P],
                                wv_sb[k][:, o:o + 512],
                                start=(k == 0),
                                stop=False,
                            )
                        nc.tensor.matmul(
                            pv[:],
                            ones_sb[:, s * P:(s + 1) * P],
                            wvb_sb[:, o:o + 512],
                            start=False,
                            stop=True,
                        )
                        if half == 0:
                            nc.vector.memset(v_sb[s][:, :, D:D + 1], 1.0)
                        nc.vector.tensor_copy(
                            v_sb[s][:, half * 8:(half + 1) * 8, 0:D], pv[:]
                        )
                return chain

            def q_chain(c, q):
                def chain():
                    pq = work.tile([P, 512], f32, tag="w", name=f"pq{_rep}_{c}_{q}")
                    for k in range(KT):
                        nc.tensor.matmul(
                            pq[:],
                            wqk_sb[k][:, c * P:(c + 1) * P],
                            xt_sb[k][:, q * 512:(q + 1) * 512],
                            start=(k == 0),
                            stop=(k == KT - 1),
                        )
                    nc.vector.tensor_scalar_add(
                        qkq_sb[c][:, q * 512:(q + 1) * 512],
                        pq[:],
                        bias_sb[:, c:c + 1],
                    )
                return chain

            def k_chain(c, q):
                def chain():
                    pq = work.tile([P, 512], f32, tag="w", name=f"pk{_rep}_{c}_{q}")
                    for k in range(KT):
                        nc.tensor.matmul(
                            pq[:],
                            wqk_sb[k][:, E + c * P:E + (c + 1) * P],
                            xt_sb[k][:, q * 512:(q + 1) * 512],
                            start=(k == 0),
                            stop=(k == KT - 1),
                        )
                    nc.vector.tensor_scalar_add(
                        qkk_sb[c][:, q * 512:(q + 1) * 512],
                        pq[:],
                        bias_sb[:, 8 + c:9 + c],
                    )
                return chain

            def qk_pair_chains(p):
                return [q_chain(p, q) for q in range(QC)] + [
                    k_chain(p, q) for q in range(KC)
                ]

            def emit_qk_pair(p):
                for ch in qk_pair_chains(p):
                    ch()

            def emit_head(h, fillers=(), stride=2):
                fillers = list(fillers)
                p, half = h // 2, h % 2
                r0 = half * D
                qT = qkq_sb[p]
                kT = qkk_sb[p]
                po = acc.tile([D + 1, SQ], f32, tag="a", name=f"po{_rep}_{h}")
                for kb in range(SB):
                    if fillers and (stride == 1 or kb % stride == 1):
                        fillers.pop(0)()
                    pt = ptp.tile([P, SQ], bf16, tag="pt", name=f"pt{_rep}_{h}_{kb}")
                    for q2 in range(QC):
                        ps = work.tile(
                            [P, 512], f32, tag="w", name=f"ps{_rep}_{h}_{kb}_{q2}"
                        )
                        nc.tensor.matmul(
                            ps[:],
                            kT[r0:r0 + D, kb * P:(kb + 1) * P],
                            qT[r0:r0 + D, q2 * 512:(q2 + 1) * 512],
                            start=True,
                            stop=True,
                        )
                        nc.scalar.activation(
                            pt[:, q2 * 512:(q2 + 1) * 512],
                            ps[:],
                            mybir.ActivationFunctionType.Exp,
                            scale=0.125,
                        )
                        nc.tensor.matmul(
                            po[:, q2 * 512:(q2 + 1) * 512],
                            v_sb[kb][:, h, :],
                            pt[:, q2 * 512:(q2 + 1) * 512],
                            start=(kb == 0),
                            stop=(kb == SB - 1),
                        )
                for ch in fillers:
                    ch()
                # normalize: O^T[0:D] / l (l = row D), write bf16
                lsb = recp.tile([1, SQ], f32, tag="lsb", name=f"lsb{_rep}_{h}")
                nc.vector.reciprocal(lsb[:], po[D:D + 1, :])
                lscr = dram.tile([1, SQ], f32, tag="lscr", name=f"lscr{_rep}_{h}", bufs=2)
                nc.sync.dma_start(out=lscr[:], in_=lsb[:])
                ldr = lscr[:]
                lbc = bass.AP(
                    tensor=ldr.tensor,
                    offset=ldr.offset,
                    ap=[[0, D]] + [list(x) for x in ldr.ap[1:]],
                )
                rin = recp.tile([D, SQ], f32, tag="rin", name=f"rin{_rep}_{h}")
                nc.sync.dma_start(out=rin[:], in_=lbc)
                nc.vector.tensor_mul(
                    on_sb[p][r0:r0 + D, :],
                    po[0:D, :],
                    rin[:],
                )

            def emit_pair(p, fillers=(), stride=2):
                # Heads (2p, 2p+1) interleaved per key-block: their K=64 score
                # matmuls sit at base partitions 0 and 64, so the PE runs them
                # concurrently in disjoint row-groups (tile_position
                # auto-derives from the lhsT base partition).
                fillers = list(fillers)
                qT = qkq_sb[p]
                kT = qkk_sb[p]
                po = [
                    acc.tile([D + 1, SQ], f32, tag="a", name=f"po{_rep}_{p}_{i}")
                    for i in range(2)
                ]
                for kb in range(SB):
                    if fillers and (stride == 1 or kb % stride == 1):
                        fillers.pop(0)()
                    pt = [
                        ptp.tile([P, SQ], bf16, tag="pt", name=f"pt{_rep}_{p}_{kb}_{i}")
                        for i in range(2)
                    ]
                    for q2 in range(QC):
                        pss = []
                        for i in range(2):
                            r0 = i * D
                            ps = work.tile(
                                [P, 512], f32, tag="w",
                                name=f"ps{_rep}_{p}_{kb}_{q2}_{i}",
                            )
                            nc.tensor.matmul(
                                ps[:],
                                kT[r0:r0 + D, kb * P:(kb + 1) * P],
                                qT[r0:r0 + D, q2 * 512:(q2 + 1) * 512],
                                start=True,
                                stop=True,
                            )
                            pss.append(ps)
                        for i in range(2):
                            nc.scalar.activation(
                                pt[i][:, q2 * 512:(q2 + 1) * 512],
                                pss[i][:],
                                mybir.ActivationFunctionType.Exp,
                                scale=0.125,
                            )
                        for i in range(2):
                            nc.tensor.matmul(
                                po[i][:, q2 * 512:(q2 + 1) * 512],
                                v_sb[kb][:, 2 * p + i, :],
                                pt[i][:, q2 * 512:(q2 + 1) * 512],
                                start=(kb == 0),
                                stop=(kb == SB - 1),
                            )
                for ch in fillers:
                    ch()
                # normalize: O^T[0:D] / l (l = row D), write bf16
                for i in range(2):
                    r0 = i * D
                    lsb = recp.tile([1, SQ], f32, tag="lsb", name=f"lsb{_rep}_{p}_{i}")
                    nc.vector.reciprocal(lsb[:], po[i][D:D + 1, :])
                    lscr = dram.tile(
                        [1, SQ], f32, tag="lscr", name=f"lscr{_rep}_{p}_{i}", bufs=2
                    )
                    nc.sync.dma_start(out=lscr[:], in_=lsb[:])
                    ldr = lscr[:]
                    lbc = bass.AP(
                        tensor=ldr.tensor,
                        offset=ldr.offset,
                        ap=[[0, D]] + [list(x) for x in ldr.ap[1:]],
                    )
                    rin = recp.tile([D, SQ], f32, tag="rin", name=f"rin{_rep}_{p}_{i}")
                    nc.sync.dma_start(out=rin[:], in_=lbc)
                    nc.vector.tensor_mul(
                        on_sb[p][r0:r0 + D, :],
                        po[i][0:D, :],
                        rin[:],
                    )

            if _pair:
                # QK pair 0 first so ScalarE starts ASAP; the 16 V chains ride
                # as per-kb fillers inside pair 0 (v_sb[kb] lands just in time
                # for pair 0's PV at kb); pair p also hosts pair p+1's QK
                # chains as fillers.
                emit_qk_pair(0)
                emit_pair(
                    0,
                    fillers=[v_chain(s) for s in range(SB)] + qk_pair_chains(1),
                    stride=1,
                )
                for p in range(1, PAIRS):
                    emit_pair(p, fillers=qk_pair_chains(p + 1) if p + 1 < PAIRS else ())
            elif _order == "fillv":
                emit_qk_pair(0)
                emit_head(
                    0,
                    fillers=[v_chain(s) for s in range(SB)],
                    stride=1,
                )
                emit_head(1, fillers=qk_pair_chains(1))
                for p in range(1, PAIRS - 1):
                    emit_head(2 * p)
                    emit_head(2 * p + 1, fillers=qk_pair_chains(p + 1))
                emit_head(2 * PAIRS - 2)
                emit_head(2 * PAIRS - 1)
            else:  # "safe"
                for ch in [v_chain(s) for s in range(SB)]:
                    ch()
                emit_qk_pair(0)
                emit_head(0)
                emit_head(1, fillers=qk_pair_chains(1))
                for p in range(1, PAIRS - 1):
                    emit_head(2 * p)
                    emit_head(2 * p + 1, fillers=qk_pair_chains(p + 1))
                emit_head(2 * PAIRS - 2)
                emit_head(2 * PAIRS - 1)

            # ---- phase 3: output projection (one DMA per e-block) ----
            for e in range(E // P):
                ye = yp.tile([P, SQ], bf16, tag="ysb", name=f"ye{_rep}_{e}")
                for q in range(QC):
                    py = work.tile([P, 512], f32, tag="w", name=f"py{_rep}_{e}_{q}")
                    for p in range(PAIRS):
                        nc.tensor.matmul(
                            py[:],
                            wo_sb[p][:, e * P:(e + 1) * P],
                            on_sb[p][:, q * 512:(q + 1) * 512],
                            start=(p == 0),
                            stop=(p == PAIRS - 1),
                        )
                    nc.vector.tensor_scalar_add(
                        ye[:, q * 512:(q + 1) * 512], py[:], bias_sb[:, 16 + e:17 + e]
                    )
                nc.sync.dma_start(out=out_d[e * P:(e + 1) * P, :], in_=ye[:])

            if _taps:
                nc.sync.dma_start(out=tap_q.ap(), in_=qkq_sb[0][:])
                nc.sync.dma_start(out=tap_k.ap(), in_=qkk_sb[0][:])
                nc.sync.dma_start(
                    out=tap_v.ap(),
                    in_=v_sb[0].rearrange("p h d -> p (h d)"),
                )
                nc.sync.dma_start(out=tap_on.ap(), in_=on_sb[0][:])

            acc_cm.__exit__(None, None, None)
            work_cm.__exit__(None, None, None)

    _split_drain_waits(nc, mybir)
    return nc


def _host_shards(x, Wqkv=None, bqkv=None, Wo=None, bo=None):
    """Per-core inputs: only xt (x[b]^T, key-halves swapped on odd cores)."""
    x = np.asarray(x, np.float32)
    in_maps = []
    for c in range(NCORES):
        b, h = c // 2, c % 2
        xt = np.ascontiguousarray(x[b].T).astype(_BF16)
        if h == 1:
            xt = np.concatenate([xt[:, SQ:], xt[:, :SQ]], axis=1)
        in_maps.append({"xt": np.ascontiguousarray(xt)})
    return in_maps


def _get_runner(consts):
    """Build the Bass program once (weights baked in as NEFF constants) and
    wrap it in a cached 8-core jitted callable.  Rebuilds if the weights
    change (the constants are baked into the NEFF)."""
    import zlib
    key = tuple(
        (a.shape, a.dtype.str, zlib.crc32(np.ascontiguousarray(a).view(np.uint8)))
        for a in consts
    )
    if "runner" in _cached and _cached.get("consts_key") == key:
        return _cached["runner"]
    _cached.pop("runner", None)
    _cached["consts_key"] = key

    import jax
    from jax.sharding import Mesh, PartitionSpec, NamedSharding
    from jax.experimental.shard_map import shard_map
    from concourse import bass2jax, mybir

    nc = _build_program(consts)
    _cached["nc"] = nc
    bass2jax.install_neuronx_cc_hook()

    partition_name = nc.partition_id_tensor.name if nc.partition_id_tensor else None
    in_names, out_names, out_avals = [], [], []
    for alloc in nc.m.functions[0].allocations:
        if not isinstance(alloc, mybir.MemoryLocationSet):
            continue
        if alloc.kind not in ("ExternalInput", "ExternalOutput"):
            continue
        name = alloc.memorylocations[0].name
        if alloc.kind == "ExternalInput":
            if name != partition_name:
                in_names.append(name)
        elif alloc.kind == "ExternalOutput":
            out_names.append(name)
            out_avals.append(
                jax.core.ShapedArray(tuple(alloc.tensor_shape), mybir.dt.np(alloc.dtype))
            )
    n_params = len(in_names)
    all_in_names = list(in_names) + list(out_names)
    if partition_name is not None:
        all_in_names.append(partition_name)

    def _body(*args):
        operands = list(args)
        if partition_name is not None:
            operands.append(bass2jax.partition_id_tensor())
        outs = bass2jax._bass_exec_p.bind(
            *operands,
            out_avals=tuple(out_avals),
            in_names=tuple(all_in_names),
            out_names=tuple(out_names),
            lowering_input_output_aliases=(),
            sim_require_finite=True,
            sim_require_nnan=True,
            nc=nc,
        )
        return tuple(outs)

    import os as _os
    devices = jax.devices()[:NCORES]
    mesh = Mesh(np.asarray(devices), ("core",))
    in_specs = (PartitionSpec("core"),) * (n_params + len(out_names))
    out_specs = (PartitionSpec("core"),) * len(out_names)
    sharding = NamedSharding(mesh, PartitionSpec("core"))
    zero_shapes = [
        ((NCORES * a.shape[0],) + tuple(a.shape[1:]), a.dtype) for a in out_avals
    ]
    donate = _os.environ.get("K_DONATE", "1") == "1"
    fastd = _os.environ.get("K_FASTD", "1") == "1"
    sharded = shard_map(
        _body, mesh=mesh, in_specs=in_specs, out_specs=out_specs, check_rep=False
    )
    donate_argnums = (
        tuple(range(n_params, n_params + len(out_names))) if donate else ()
    )
    if fastd:
        # lower with abstract avals so the compile happens inside the
        # fast-dispatch context (C++ no-effect dispatch path)
        in_sds = []
        for alloc_name in in_names:
            for alloc in nc.m.functions[0].allocations:
                if not isinstance(alloc, mybir.MemoryLocationSet):
                    continue
                if alloc.memorylocations[0].name == alloc_name:
                    shp = tuple(alloc.tensor_shape)
                    in_sds.append(
                        jax.ShapeDtypeStruct(
                            (NCORES * shp[0],) + shp[1:],
                            mybir.dt.np(alloc.dtype),
                            sharding=sharding,
                        )
                    )
                    break
        for shp, dt in zero_shapes:
            in_sds.append(jax.ShapeDtypeStruct(shp, dt, sharding=sharding))
        jitted = bass2jax.fast_dispatch_compile(
            lambda: jax.jit(
                sharded, donate_argnums=donate_argnums, keep_unused=True
            ).lower(*in_sds).compile()
        )
    else:
        jitted = jax.jit(
            sharded, donate_argnums=donate_argnums, keep_unused=True
        )

    def run(in_maps):
        concat_in = [
            np.concatenate([np.asarray(in_maps[c][nm]) for c in range(NCORES)], axis=0)
            for nm in in_names
        ]
        args = [jax.device_put(a, sharding) for a in concat_in] + [
            jax.device_put(np.zeros(shp, dt), sharding) for shp, dt in zero_shapes
        ]
        outs = jitted(*args)
        outs = [np.asarray(o) for o in outs]
        per_core = [
            {
                nm: outs[i].reshape(NCORES, *out_avals[i].shape)[c]
                for i, nm in enumerate(out_names)
            }
            for c in range(NCORES)
        ]
        return per_core

    _cached["runner"] = run
    _cached["jitted"] = jitted
    _cached["meta"] = (in_names, out_names, out_avals, sharding)
    _cached["donate"] = donate
    return run


def kernel(x, Wqkv, bqkv, Wo, bo):
    run = _get_runner(_weight_consts(Wqkv, bqkv, Wo, bo))
    in_maps = _host_shards(x)
    results = run(in_maps)

    out = np.empty((B, S, E), np.float32)
    for c in range(NCORES):
        b, h = c // 2, c % 2
        out[b, h * SQ:(h + 1) * SQ, :] = results[c]["out"].T
    return out


# revision 38
# speedup vs baseline: 2.5021x; 1.0939x over previous
"""Multi-head attention (B=4, S=2048, E=1024, H=16, D=64) on 8 TRN2 NeuronCores.

Sharding: data-parallel over batch (4) x sequence-parallel over queries (2).
Core c handles batch c//2 and query half c%2 (1024 queries), ALL 16 heads.
K/V are computed redundantly per core from the full (replicated) x[b], so no
cross-core reduction or collective is needed anywhere.  The host rotates the
key sequence for odd cores (swap halves) so each core's queries are always
columns 0..1023 of its xt — the program stays SPMD-uniform, and attention is
permutation-invariant over keys.

Weights/biases are identical on every core, so they are baked into the NEFF
as Const tensors (DMA'd to HBM once at model load).  The only per-execute
I/O is xt [1024, 2048] bf16 in and y^T [1024, 1024] f32 out — per-execute
dispatch cost through the PJRT path scales with I/O bytes/buffers.

Per-core dataflow (matmuls bf16 inputs, fp32 PSUM accumulation):
  phase 1: QKV projection.
    Q^T [1024, 1024] / K^T [1024, 2048] column-major via lhsT=W, rhs=x^T.
    V [2048, 16, 64] row-major via lhsT=x^T, rhs=Wv; a bias row appended to
    Wv plus an on-chip ones row implements +bias; an on-chip ones *column*
    appended to V makes the PV matmul also produce softmax row-sums.
  phase 2: per head: S^T = K^T-tiles @ Q^T (keys on partitions), exp on
    ScalarE straight from PSUM (fused 1/8 scale, bf16 out), O^T[65,1024]
    accumulated over 16 key-blocks where row 64 = softmax denominator l.
    Normalize: partition-broadcast DMA of 1/l, DVE multiply (fused bf16).
  phase 3: out-proj y^T[1024,1024] = Wo-tiles^T @ O^T (+bias), DMA to HBM.
"""

import numpy as np
import ml_dtypes

B, S, E, H, D = 4, 2048, 1024, 16, 64
NCORES = 8
SQ = S // 2            # queries per core = 1024
P = 128
KT = E // P            # 8 contraction tiles over E
SB = S // P            # 16 key blocks of 128
PAIRS = H // 2         # 8 head pairs (2 heads share a 128-row block)
QC = SQ // 512         # 2 query chunks of 512
KC = S // 512          # 4 key-dim chunks of 512 (for K^T projection)

_BF16 = ml_dtypes.bfloat16

_cached = {}


_MULTIWAIT_OK = ("InstMatmult", "InstActivation", "InstTensor", "InstCopy",
                 "InstMemSet", "InstReciprocal", "InstLdweights")


def _split_drain_waits(nc, mybir, max_waits=1, compute_max_waits=1):
    """This walrus build rejects >1 sem wait per instruction (setupSyncWait
    S3D3_TS fires even for DVE/ACT ops); hoist extras onto preceding
    same-engine nops."""
    for f in nc.m.functions:
        for bb in f.blocks:
            insts = bb.instructions
            i = 0
            while i < len(insts):
                inst = insts[i]
                si = inst.sync_info
                lim = (
                    compute_max_waits
                    if type(inst).__name__.startswith(_MULTIWAIT_OK)
                    else max_waits
                )
                if si is not None and len(si.on_wait) > lim:
                    extra = list(si.on_wait[lim:])
                    keep = list(si.on_wait[:lim])
                    for j, w in enumerate(extra):
                        nop = mybir.InstNoOp(
                            name=f"{inst.name}-waitsplit{j}", ins=[], outs=[]
                        )
                        nop.engine = inst.engine
                        nop.sync_info = mybir.SyncInfo(on_wait=[w], on_update=[])
                        nc.register_instruction(nop)
                        insts.insert(i, nop)
                        i += 1
                    inst.sync_info = mybir.SyncInfo(
                        on_wait=keep, on_update=list(si.on_update)
                    )
                i += 1


def _weight_consts(Wqkv, bqkv, Wo, bo):
    """Host-side packing of the (core-invariant) weight constants."""
    Wqkv = np.asarray(Wqkv, np.float32)
    bqkv = np.asarray(bqkv, np.float32)
    Wo = np.asarray(Wo, np.float32)
    bo = np.asarray(bo, np.float32)

    wqk = np.ascontiguousarray(Wqkv[:, 0:2 * E]).astype(_BF16)   # [E, 2E] Wq|Wk
    wv = np.empty((E + 1, E), _BF16)                             # Wv + bias row
    wv[0:E] = Wqkv[:, 2 * E:3 * E].astype(_BF16)
    wv[E] = bqkv[2 * E:3 * E].astype(_BF16)
    wo = Wo.astype(_BF16)                                        # [CS, E]
    bias = np.empty((P, 24), np.float32)
    bias[:, 0:8] = bqkv[0:E].reshape(8, P).T                     # Q bias blocks
    bias[:, 8:16] = bqkv[E:2 * E].reshape(8, P).T                # K bias blocks
    bias[:, 16:24] = bo.reshape(8, P).T                          # out bias blocks
    return wqk, wv, wo, bias


def _build_program(consts):
    import os as _os
    import concourse.bass as bass
    import concourse.tile as tile
    from concourse import mybir

    f32 = mybir.dt.float32
    bf16 = mybir.dt.bfloat16

    wqk_np, wv_np, wo_np, bias_np = consts

    nc = bass.Bass("TRN2", num_devices=NCORES, debug=False)

    xt_d = nc.dram_tensor("xt", [E, S], bf16, kind="ExternalInput")
    out_d = nc.dram_tensor("out", [E, SQ], bf16, kind="ExternalOutput")
    wqk_d = nc.inline_tensor(wqk_np, name="wqk_c")
    wv_d = nc.inline_tensor(wv_np, name="wv_c")
    wo_d = nc.inline_tensor(wo_np, name="wo_c")
    bias_d = nc.inline_tensor(bias_np, name="bias_c")

    _taps = _os.environ.get("K_TAPS") == "1"
    if _taps:
        tap_q = nc.dram_tensor("tap_q", [P, SQ], bf16, kind="ExternalOutput")
        tap_k = nc.dram_tensor("tap_k", [P, S], bf16, kind="ExternalOutput")
        tap_v = nc.dram_tensor("tap_v", [P, H * (D + 1)], bf16, kind="ExternalOutput")
        tap_on = nc.dram_tensor("tap_on", [P, SQ], bf16, kind="ExternalOutput")

    with tile.TileContext(nc) as tc:
        with (
            tc.tile_pool(name="const", bufs=1) as const,
            tc.tile_pool(name="xtw", bufs=1) as xtw,
            tc.tile_pool(name="wvon", bufs=1) as wvon,
            tc.tile_pool(name="qk", bufs=1) as qkp,
            tc.tile_pool(name="vsb", bufs=1) as vp,
            tc.tile_pool(name="pt", bufs=int(__import__("os").environ.get("K_PT", "4"))) as ptp,
            tc.tile_pool(name="rec", bufs=int(__import__("os").environ.get("K_REC", "2"))) as recp,
            tc.tile_pool(name="ysb", bufs=2) as yp,
            tc.tile_pool(name="dram", bufs=1, space="DRAM") as dram,
        ):
          _repeat = int(_os.environ.get("K_REPEAT", "1"))
          _order = _os.environ.get("K_ORDER", "fillv")
          for _rep in range(_repeat):
            # ---- constants / weights into SBUF ----
            bias_sb = const.tile([P, 24], f32, tag="bias")
            nc.sync.dma_start(out=bias_sb[:], in_=bias_d.ap())
            ones_sb = const.tile([1, S], bf16, tag="ones")
            nc.vector.memset(ones_sb[:], 1.0)

            # x^T [128, 2048] and [Wq|Wk] [128, 2048] tiles, interleaved so
            # the first QK chains can start as soon as their tiles land
            xt_sb = [
                xtw.tile([P, S], bf16, tag=f"xt{k}", name=f"xt{_rep}_{k}")
                for k in range(KT)
            ]
            wqk_sb = [
                xtw.tile([P, S], bf16, tag=f"wqk{k}", name=f"wqk{_rep}_{k}")
                for k in range(KT)
            ]
            for k in range(KT):
                nc.sync.dma_start(out=xt_sb[k][:], in_=xt_d[k * P:(k + 1) * P, :])
                nc.sync.dma_start(out=wqk_sb[k][:], in_=wqk_d[k * P:(k + 1) * P, :])

            wv_sb = [
                wvon.tile([P, E], bf16, tag=f"wv{k}", name=f"wv{_rep}_{k}")
                for k in range(KT)
            ]
            for k in range(KT):
                nc.sync.dma_start(out=wv_sb[k][:], in_=wv_d[k * P:(k + 1) * P, :])
            wvb_sb = const.tile([1, E], bf16, tag="wvb")
            nc.sync.dma_start(out=wvb_sb[:], in_=wv_d[E:E + 1, :])

            # Wo tiles reuse the xt slots (xt dead once the last QK chain ran)
            wo_sb = [
                xtw.tile([P, E], bf16, tag=f"xt{p}", name=f"wo{_rep}_{p}")
                for p in range(PAIRS)
            ]
            for p in range(PAIRS):
                nc.sync.dma_start(out=wo_sb[p][:], in_=wo_d[p * P:(p + 1) * P, :])

            # persistent activations
            qkq_sb = [
                qkp.tile([P, SQ], bf16, tag=f"qq{c}", name=f"qq{_rep}_{c}")
                for c in range(PAIRS)
            ]
            qkk_sb = [
                qkp.tile([P, S], bf16, tag=f"qk{c}", name=f"qk{_rep}_{c}")
                for c in range(PAIRS)
            ]
            v_sb = [
                vp.tile([P, H, D + 1], bf16, tag=f"v{s}", name=f"v{_rep}_{s}")
                for s in range(SB)
            ]
            # O^T (normalized) tiles reuse the wv pool slots (wv is dead once
            # the V phase finishes; same [128, 1024] bf16 footprint)
            on_sb = [
                wvon.tile([P, SQ], bf16, tag=f"wv{p}", name=f"on{_rep}_{p}")
                for p in range(PAIRS)
            ]

            # ---- shared PSUM pools (8 banks static across all phases):
            # pair mode: work 2x1-bank + acc 3x2-bank;
            # head mode: work 4x1-bank + acc 2x2-bank;
            # wide-act mode: work 2x2-bank + acc 4x1-bank ----
            _pair = _os.environ.get("K_PAIR", "0") == "1"
            _wide = _os.environ.get("K_WIDE", "0") == "1"
            work_cm = tc.tile_pool(name="work", bufs=2 if (_pair or _wide) else 4,
                                   space="PSUM")
            work = work_cm.__enter__()
            acc_cm = tc.tile_pool(name="acc", bufs=3 if _pair else (4 if _wide else 2),
                                  space="PSUM")
            acc = acc_cm.__enter__()

            def v_chain(s):
                # V: out[key-block s, 1024 head-dims]; lhsT = x^T tile, rhs = Wv
                def chain():
                    for half in range(2):
                        o = half * 512
                        pv = work.tile(
                            [P, 512], f32, tag="w", name=f"pv{_rep}_{s}_{half}"
                        )
                        for k in range(KT):
                            nc.tensor.matmul(
                                pv[:],
                                xt_sb[k][:, s * P:(s + 1) * P],
                                wv_sb[k][:, o:o + 512],
                                start=(k == 0),
                                stop=False,
                            )
                        nc.tensor.matmul(
                            pv[:],
                            ones_sb[:, s * P:(s + 1) * P],
                            wvb_sb[:, o:o + 512],
                            start=False,
                            stop=True,
                        )
                        if half == 0:
                            nc.vector.memset(v_sb[s][:, :, D:D + 1], 1.0)
                        nc.vector.tensor_copy(
                            v_sb[s][:, half * 8:(half + 1) * 8, 0:D], pv[:]
                        )
                return chain

            def q_chain(c, q):
                def chain():
                    pq = work.tile([P, 512], f32, tag="w", name=f"pq{_rep}_{c}_{q}")
                    for k in range(KT):
                        nc.tensor.matmul(
                            pq[:],
                            wqk_sb[k][:, c * P:(c + 1) * P],
                            xt_sb[k][:, q * 512:(q + 1) * 512],
                            start=(k == 0),
                            stop=(k == KT - 1),
                        )
                    nc.vector.tensor_scalar_add(
                        qkq_sb[c][:, q * 512:(q + 1) * 512],
                        pq[:],
                        bias_sb[:, c:c + 1],
                    )
                return chain

            def k_chain(c, q):
                def chain():
                    pq = work.tile([P, 512], f32, tag="w", name=f"pk{_rep}_{c}_{q}")
                    for k in range(KT):
                        nc.tensor.matmul(
                            pq[:],
                            wqk_sb[k][:, E + c * P:E + (c + 1) * P],
                            xt_sb[k][:, q * 512:(q + 1) * 512],
                            start=(k == 0),
                            stop=(k == KT - 1),
                        )
                    nc.vector.tensor_scalar_add(
                        qkk_sb[c][:, q * 512:(q + 1) * 512],
                        pq[:],
                        bias_sb[:, 8 + c:9 + c],
                    )
                return chain

            def qk_pair_chains(p):
                return [q_chain(p, q) for q in range(QC)] + [
                    k_chain(p, q) for q in range(KC)
                ]

            def emit_qk_pair(p):
                for ch in qk_pair_chains(p):
                    ch()

            def emit_head(h, fillers=(), stride=2):
                fillers = list(fillers)
                p, half = h // 2, h % 2
                r0 = half * D
                qT = qkq_sb[p]
                kT = qkk_sb[p]
                if _wide:
                    # one [65, 512] po per query chunk (1 bank each); one
                    # N=1024 activation per key block spanning both banks of
                    # a [128, 1024] score tile
                    po2 = [
                        acc.tile([D + 1, 512], f32, tag="a",
                                 name=f"po{_rep}_{h}_{q2}")
                        for q2 in range(QC)
                    ]
                else:
                    po = acc.tile([D + 1, SQ], f32, tag="a", name=f"po{_rep}_{h}")
                for kb in range(SB):
                    if fillers and (stride == 1 or kb % stride == 1):
                        fillers.pop(0)()
                    pt = ptp.tile([P, SQ], bf16, tag="pt", name=f"pt{_rep}_{h}_{kb}")
                    if _wide:
                        ps = work.tile(
                            [P, SQ], f32, tag="w", name=f"ps{_rep}_{h}_{kb}"
                        )
                        for q2 in range(QC):
                            nc.tensor.matmul(
                                ps[:, q2 * 512:(q2 + 1) * 512],
                                kT[r0:r0 + D, kb * P:(kb + 1) * P],
                                qT[r0:r0 + D, q2 * 512:(q2 + 1) * 512],
                                start=True,
                                stop=True,
                            )
                        nc.scalar.activation(
                            pt[:],
                            ps[:],
                            mybir.ActivationFunctionType.Exp,
                            scale=0.125,
                        )
                        for q2 in range(QC):
                            nc.tensor.matmul(
                                po2[q2][:],
                                v_sb[kb][:, h, :],
                                pt[:, q2 * 512:(q2 + 1) * 512],
                                start=(kb == 0),
                                stop=(kb == SB - 1),
                            )
                    else:
                        for q2 in range(QC):
                            ps = work.tile(
                                [P, 512], f32, tag="w", name=f"ps{_rep}_{h}_{kb}_{q2}"
                            )
                            nc.tensor.matmul(
                                ps[:],
                                kT[r0:r0 + D, kb * P:(kb + 1) * P],
                                qT[r0:r0 + D, q2 * 512:(q2 + 1) * 512],
                                start=True,
                                stop=True,
                            )
                            nc.scalar.activation(
                                pt[:, q2 * 512:(q2 + 1) * 512],
                                ps[:],
                                mybir.ActivationFunctionType.Exp,
                                scale=0.125,
                            )
                            nc.tensor.matmul(
                                po[:, q2 * 512:(q2 + 1) * 512],
                                v_sb[kb][:, h, :],
                                pt[:, q2 * 512:(q2 + 1) * 512],
                                start=(kb == 0),
                                stop=(kb == SB - 1),
                            )
                for ch in fillers:
                    ch()
                # normalize: O^T[0:D] / l (l = row D), write bf16
                lsb = recp.tile([1, SQ], f32, tag="lsb", name=f"lsb{_rep}_{h}")
                if _wide:
                    for q2 in range(QC):
                        nc.vector.reciprocal(
                            lsb[:, q2 * 512:(q2 + 1) * 512], po2[q2][D:D + 1, :]
                        )
                else:
                    nc.vector.reciprocal(lsb[:], po[D:D + 1, :])
                lscr = dram.tile([1, SQ], f32, tag="lscr", name=f"lscr{_rep}_{h}", bufs=2)
                nc.sync.dma_start(out=lscr[:], in_=lsb[:])
                ldr = lscr[:]
                lbc = bass.AP(
                    tensor=ldr.tensor,
                    offset=ldr.offset,
                    ap=[[0, D]] + [list(x) for x in ldr.ap[1:]],
                )
                rin = recp.tile([D, SQ], f32, tag="rin", name=f"rin{_rep}_{h}")
                nc.sync.dma_start(out=rin[:], in_=lbc)
                if _wide:
                    for q2 in range(QC):
                        nc.vector.tensor_mul(
                            on_sb[p][r0:r0 + D, q2 * 512:(q2 + 1) * 512],
                            po2[q2][0:D, :],
                            rin[:, q2 * 512:(q2 + 1) * 512],
                        )
                else:
                    nc.vector.tensor_mul(
                        on_sb[p][r0:r0 + D, :],
                        po[0:D, :],
                        rin[:],
                    )

            def emit_pair(p, fillers=(), stride=2):
                # Heads (2p, 2p+1) interleaved per key-block: their K=64 score
                # matmuls sit at base partitions 0 and 64, so the PE runs them
                # concurrently in disjoint row-groups (tile_position
                # auto-derives from the lhsT base partition).
                fillers = list(fillers)
                qT = qkq_sb[p]
                kT = qkk_sb[p]
                po = [
                    acc.tile([D + 1, SQ], f32, tag="a", name=f"po{_rep}_{p}_{i}")
                    for i in range(2)
                ]
                for kb in range(SB):
                    if fillers and (stride == 1 or kb % stride == 1):
                        fillers.pop(0)()
                    pt = [
                        ptp.tile([P, SQ], bf16, tag="pt", name=f"pt{_rep}_{p}_{kb}_{i}")
                        for i in range(2)
                    ]
                    for q2 in range(QC):
                        pss = []
                        for i in range(2):
                            r0 = i * D
                            ps = work.tile(
                                [P, 512], f32, tag="w",
                                name=f"ps{_rep}_{p}_{kb}_{q2}_{i}",
                            )
                            nc.tensor.matmul(
                                ps[:],
                                kT[r0:r0 + D, kb * P:(kb + 1) * P],
                                qT[r0:r0 + D, q2 * 512:(q2 + 1) * 512],
                                start=True,
                                stop=True,
                            )
                            pss.append(ps)
                        for i in range(2):
                            nc.scalar.activation(
                                pt[i][:, q2 * 512:(q2 + 1) * 512],
                                pss[i][:],
                                mybir.ActivationFunctionType.Exp,
                                scale=0.125,
                            )
                        for i in range(2):
                            nc.tensor.matmul(
                                po[i][:, q2 * 512:(q2 + 1) * 512],
                                v_sb[kb][:, 2 * p + i, :],
                                pt[i][:, q2 * 512:(q2 + 1) * 512],
                                start=(kb == 0),
                                stop=(kb == SB - 1),
                            )
                for ch in fillers:
                    ch()
                # normalize: O^T[0:D] / l (l = row D), write bf16
                for i in range(2):
                    r0 = i * D
                    lsb = recp.tile([1, SQ], f32, tag="lsb", name=f"lsb{_rep}_{p}_{i}")
                    nc.vector.reciprocal(lsb[:], po[i][D:D + 1, :])
                    lscr = dram.tile(
                        [1, SQ], f32, tag="lscr", name=f"lscr{_rep}_{p}_{i}", bufs=2
                    )
                    nc.sync.dma_start(out=lscr[:], in_=lsb[:])
                    ldr = lscr[:]
                    lbc = bass.AP(
                        tensor=ldr.tensor,
                        offset=ldr.offset,
                        ap=[[0, D]] + [list(x) for x in ldr.ap[1:]],
                    )
                    rin = recp.tile([D, SQ], f32, tag="rin", name=f"rin{_rep}_{p}_{i}")
                    nc.sync.dma_start(out=rin[:], in_=lbc)
                    nc.vector.tensor_mul(
                        on_sb[p][r0:r0 + D, :],
                        po[i][0:D, :],
                        rin[:],
                    )

            if _pair:
                # QK pair 0 first so ScalarE starts ASAP; the 16 V chains ride
                # as per-kb fillers inside pair 0 (v_sb[kb] lands just in time
                # for pair 0's PV at kb); pair p also hosts pair p+1's QK
                # chains as fillers.
                emit_qk_pair(0)
                emit_pair(
                    0,
                    fillers=[v_chain(s) for s in range(SB)] + qk_pair_chains(1),
                    stride=1,
                )
                for p in range(1, PAIRS):
                    emit_pair(p, fillers=qk_pair_chains(p + 1) if p + 1 < PAIRS else ())
            elif _order == "fillv":
                emit_qk_pair(0)
                emit_head(
                    0,
                    fillers=[v_chain(s) for s in range(SB)],
                    stride=1,
                )
                emit_head(1, fillers=qk_pair_chains(1))
                for p in range(1, PAIRS - 1):
                    emit_head(2 * p)
                    emit_head(2 * p + 1, fillers=qk_pair_chains(p + 1))
                emit_head(2 * PAIRS - 2)
                emit_head(2 * PAIRS - 1)
            else:  # "safe"
                for ch in [v_chain(s) for s in range(SB)]:
                    ch()
                emit_qk_pair(0)
                emit_head(0)
                emit_head(1, fillers=qk_pair_chains(1))
                for p in range(1, PAIRS - 1):
                    emit_head(2 * p)
                    emit_head(2 * p + 1, fillers=qk_pair_chains(p + 1))
                emit_head(2 * PAIRS - 2)
                emit_head(2 * PAIRS - 1)

            # ---- phase 3: output projection (one DMA per e-block) ----
            for e in range(E // P):
                ye = yp.tile([P, SQ], bf16, tag="ysb", name=f"ye{_rep}_{e}")
                for q in range(QC):
                    py = work.tile([P, 512], f32, tag="w", name=f"py{_rep}_{e}_{q}")
                    for p in range(PAIRS):
                        nc.tensor.matmul(
                            py[:],
                            wo_sb[p][:, e * P:(e + 1) * P],
                            on_sb[p][:, q * 512:(q + 1) * 512],
                            start=(p == 0),
                            stop=(p == PAIRS - 1),
                        )
                    nc.vector.tensor_scalar_add(
                        ye[:, q * 512:(q + 1) * 512], py[:], bias_sb[:, 16 + e:17 + e]
                    )
                nc.sync.dma_start(out=out_d[e * P:(e + 1) * P, :], in_=ye[:])

            if _taps:
                nc.sync.dma_start(out=tap_q.ap(), in_=qkq_sb[0][:])
                nc.sync.dma_start(out=tap_k.ap(), in_=qkk_sb[0][:])
                nc.sync.dma_start(
                    out=tap_v.ap(),
                    in_=v_sb[0].rearrange("p h d -> p (h d)"),
                )
                nc.sync.dma_start(out=tap_on.ap(), in_=on_sb[0][:])

            acc_cm.__exit__(None, None, None)
            work_cm.__exit__(None, None, None)

    _split_drain_waits(nc, mybir)
    return nc


def _host_shards(x, Wqkv=None, bqkv=None, Wo=None, bo=None):
    """Per-core inputs: only xt (x[b]^T, key-halves swapped on odd cores)."""
    x = np.asarray(x, np.float32)
    in_maps = []
    for c in range(NCORES):
        b, h = c // 2, c % 2
        xt = np.ascontiguousarray(x[b].T).astype(_BF16)
        if h == 1:
            xt = np.concatenate([xt[:, SQ:], xt[:, :SQ]], axis=1)
        in_maps.append({"xt": np.ascontiguousarray(xt)})
    return in_maps


def _get_runner(consts):
    """Build the Bass program once (weights baked in as NEFF constants) and
    wrap it in a cached 8-core jitted callable.  Rebuilds if the weights
    change (the constants are baked into the NEFF)."""
    import zlib
    key = tuple(
        (a.shape, a.dtype.str, zlib.crc32(np.ascontiguousarray(a).view(np.uint8)))
        for a in consts
    )
    if "runner" in _cached and _cached.get("consts_key") == key:
        return _cached["runner"]
    _cached.pop("runner", None)
    _cached["consts_key"] = key

    import jax
    from jax.sharding import Mesh, PartitionSpec, NamedSharding
    from jax.experimental.shard_map import shard_map
    from concourse import bass2jax, mybir

    nc = _build_program(consts)
    _cached["nc"] = nc
    bass2jax.install_neuronx_cc_hook()

    partition_name = nc.partition_id_tensor.name if nc.partition_id_tensor else None
    in_names, out_names, out_avals = [], [], []
    for alloc in nc.m.functions[0].allocations:
        if not isinstance(alloc, mybir.MemoryLocationSet):
            continue
        if alloc.kind not in ("ExternalInput", "ExternalOutput"):
            continue
        name = alloc.memorylocations[0].name
        if alloc.kind == "ExternalInput":
            if name != partition_name:
                in_names.append(name)
        elif alloc.kind == "ExternalOutput":
            out_names.append(name)
            out_avals.append(
                jax.core.ShapedArray(tuple(alloc.tensor_shape), mybir.dt.np(alloc.dtype))
            )
    n_params = len(in_names)
    all_in_names = list(in_names) + list(out_names)
    if partition_name is not None:
        all_in_names.append(partition_name)

    def _body(*args):
        operands = list(args)
        if partition_name is not None:
            operands.append(bass2jax.partition_id_tensor())
        outs = bass2jax._bass_exec_p.bind(
            *operands,
            out_avals=tuple(out_avals),
            in_names=tuple(all_in_names),
            out_names=tuple(out_names),
            lowering_input_output_aliases=(),
            sim_require_finite=True,
            sim_require_nnan=True,
            nc=nc,
        )
        return tuple(outs)

    import os as _os
    devices = jax.devices()[:NCORES]
    mesh = Mesh(np.asarray(devices), ("core",))
    in_specs = (PartitionSpec("core"),) * (n_params + len(out_names))
    out_specs = (PartitionSpec("core"),) * len(out_names)
    sharding = NamedSharding(mesh, PartitionSpec("core"))
    zero_shapes = [
        ((NCORES * a.shape[0],) + tuple(a.shape[1:]), a.dtype) for a in out_avals
    ]
    donate = _os.environ.get("K_DONATE", "1") == "1"
    fastd = _os.environ.get("K_FASTD", "1") == "1"
    sharded = shard_map(
        _body, mesh=mesh, in_specs=in_specs, out_specs=out_specs, check_rep=False
    )
    donate_argnums = (
        tuple(range(n_params, n_params + len(out_names))) if donate else ()
    )
    if fastd:
        # lower with abstract avals so the compile happens inside the
        # fast-dispatch context (C++ no-effect dispatch path)
        in_sds = []
        for alloc_name in in_names:
            for alloc in nc.m.functions[0].allocations:
                if not isinstance(alloc, mybir.MemoryLocationSet):
                    continue
                if alloc.memorylocations[0].name == alloc_name:
                    shp = tuple(alloc.tensor_shape)
                    in_sds.append(
                        jax.ShapeDtypeStruct(
                            (NCORES * shp[0],) + shp[1:],
                            mybir.dt.np(alloc.dtype),
                            sharding=sharding,
                        )
                    )
                    break
        for shp, dt in zero_shapes:
            in_sds.append(jax.ShapeDtypeStruct(shp, dt, sharding=sharding))
        jitted = bass2jax.fast_dispatch_compile(
            lambda: jax.jit(
                sharded, donate_argnums=donate_argnums, keep_unused=True
            ).lower(*in_sds).compile()
        )
    else:
        jitted = jax.jit(
            sharded, donate_argnums=donate_argnums, keep_unused=True
        )

    def run(in_maps):
        concat_in = [
            np.concatenate([np.asarray(in_maps[c][nm]) for c in range(NCORES)], axis=0)
            for nm in in_names
        ]
        args = [jax.device_put(a, sharding) for a in concat_in] + [
            jax.device_put(np.zeros(shp, dt), sharding) for shp, dt in zero_shapes
        ]
        outs = jitted(*args)
        outs = [np.asarray(o) for o in outs]
        per_core = [
            {
                nm: outs[i].reshape(NCORES, *out_avals[i].shape)[c]
                for i, nm in enumerate(out_names)
            }
            for c in range(NCORES)
        ]
        return per_core

    _cached["runner"] = run
    _cached["jitted"] = jitted
    _cached["meta"] = (in_names, out_names, out_avals, sharding)
    _cached["donate"] = donate
    return run


def kernel(x, Wqkv, bqkv, Wo, bo):
    run = _get_runner(_weight_consts(Wqkv, bqkv, Wo, bo))
    in_maps = _host_shards(x)
    results = run(in_maps)

    out = np.empty((B, S, E), np.float32)
    for c in range(NCORES):
        b, h = c // 2, c % 2
        out[b, h * SQ:(h + 1) * SQ, :] = results[c]["out"].T
    return out
